# revision 11
# baseline (speedup 1.0000x reference)
"""EIF neuron kernel for Trainium2 (Bass/Tile), 8-core data-parallel.

Reference semantics (TAU=1.0, V_TH=1.0, DELTA_T=0.2, V_RESET=0.0):
    e      = exp((mem - 1) / 0.2)
    mem'   = mem + (x_t - mem + 0.2*e) / 1.0   == x_t + 0.2*e
    spike  = (mem' >= 1)
    mem    = where(spike, 0, mem')

Sharding: batch dim B=32 -> 4 batches per core.  Per core, the
(b, n) element set is 4*4096 = 16384 = 128 partitions x 128 free.
The T=512 recurrence is serial; per timestep we process a
[128, 128] tile split into G=2 independent column groups so the
two serial chains pipeline across ACT (exp) and DVE (update).
Spikes are compared on GPSIMD in 8-step batches off the critical
chain.
"""

import numpy as np
from contextlib import ExitStack

import concourse.bass as bass
import concourse.bacc as bacc
import concourse.tile as tile
from concourse import mybir
from concourse.bass_utils import run_bass_kernel_spmd

F32 = mybir.dt.float32
AF = mybir.ActivationFunctionType
OP = mybir.AluOpType

B, T, N = 32, 512, 4096
NCORES = 8
BPC = B // NCORES            # 4 batches per core
P = 128                      # SBUF partitions
FD = (BPC * N) // P          # 128 free columns per timestep
G = 2                        # interleaved chain groups
GF = FD // G                 # 64 columns per group
TC = 64                      # timesteps per DMA chunk
RING = 8                     # timesteps per batched spike compare

V_TH = 1.0
DELTA_T = 0.2
INV_DT = 5.0                 # 1/DELTA_T
# 0.2*exp(5m - 5) == exp(5m - 5 + ln(0.2)); fold the multiplier into the bias
EXP_BIAS = -5.0 + float(np.log(0.2))

_built = None


def _build(reps=1, spike_engine="gpsimd"):
    nc = bacc.Bacc("TRN2", debug=False, num_devices=NCORES)
    x_d = nc.declare_dram_parameter("x", [P, T * FD], F32, isOutput=False)
    s_d = nc.declare_dram_parameter("spk", [P, T * FD], F32, isOutput=True)

    with ExitStack() as ctx:
        tc = ctx.enter_context(tile.TileContext(nc))
        xpool = ctx.enter_context(tc.tile_pool(name="xin", bufs=2))
        spool = ctx.enter_context(tc.tile_pool(name="sout", bufs=2))
        rpool = ctx.enter_context(tc.tile_pool(name="ring", bufs=3))
        state = ctx.enter_context(tc.tile_pool(name="state", bufs=1))

        m = [state.tile([P, GF], F32, name=f"m{g}", tag=f"m{g}") for g in range(G)]
        e = [state.tile([P, GF], F32, name=f"e{g}", tag=f"e{g}") for g in range(G)]
        bconst = state.tile([P, 1], F32, name="bconst", tag="bconst")
        nc.vector.memset(bconst[:], EXP_BIAS)
        for g in range(G):
            nc.vector.memset(m[g][:], 0.0)

        for _rep in range(reps):
            for ci in range(T // TC):
                xt = xpool.tile([P, TC * FD], F32, name="xt", tag="x")
                nc.sync.dma_start(
                    out=xt[:], in_=x_d[:, ci * TC * FD:(ci + 1) * TC * FD]
                )
                sp = spool.tile([P, TC * FD], F32, name="sp", tag="s")
                sp3 = sp.rearrange("p (t f) -> p t f", f=FD)

                for w in range(TC // RING):
                    rings = [
                        rpool.tile([P, RING * GF], F32, name=f"ring{g}", tag=f"r{g}") for g in range(G)
                    ]
                    for k in range(RING):
                        t = w * RING + k
                        for g in range(G):
                            # e = 0.2 * exp(5*m - 5)  (multiplier folded into bias)
                            nc.scalar.activation(
                                e[g][:], m[g][:], AF.Exp,
                                bias=bconst[:], scale=INV_DT,
                            )
                            mp = rings[g][:, k * GF:(k + 1) * GF]
                            xs = xt[:, t * FD + g * GF: t * FD + (g + 1) * GF]
                            # m' = e + x_t
                            nc.vector.tensor_tensor(mp, e[g][:], xs, OP.add)
                            # m = (m' < 1) * m'
                            nc.vector.scalar_tensor_tensor(
                                m[g][:], mp, V_TH, mp, OP.is_lt, OP.mult
                            )
                    for g in range(G):
                        # spikes = (m' >= 1) for the last RING steps, batched
                        dst = sp3[:, w * RING:(w + 1) * RING, g * GF:(g + 1) * GF]
                        src = rings[g].rearrange("p (t f) -> p t f", f=GF)
                        if spike_engine == "gpsimd":
                            nc.gpsimd.tensor_scalar(dst, src, V_TH, None, OP.is_ge)
                        else:
                            nc.vector.tensor_scalar(dst, src, V_TH, None, OP.is_ge)

                nc.sync.dma_start(
                    out=s_d[:, ci * TC * FD:(ci + 1) * TC * FD], in_=sp[:]
                )
    nc.compile()
    return nc


def _shard(x):
    """x[B,T,N] -> per-core [P, T*FD] partition-major arrays."""
    maps = []
    for c in range(NCORES):
        xc = x[c * BPC:(c + 1) * BPC]                      # [4, T, 4096]
        xc = np.ascontiguousarray(
            xc.reshape(BPC, T, N // FD, FD).transpose(0, 2, 1, 3)
        ).reshape(P, T * FD)
        maps.append({"x": xc})
    return maps


def _unshard(results):
    out = np.empty((B, T, N), np.float32)
    for c in range(NCORES):
        r = np.asarray(results[c]["spk"]).reshape(BPC, N // FD, T, FD)
        out[c * BPC:(c + 1) * BPC] = (
            r.transpose(0, 2, 1, 3).reshape(BPC, T, N)
        )
    return out


def kernel(x):
    global _built
    x = np.asarray(x, dtype=np.float32)
    assert x.shape == (B, T, N), x.shape
    if _built is None:
        _built = _build()
    res = run_bass_kernel_spmd(_built, _shard(x), list(range(NCORES)))
    return _unshard(res.results)


# revision 14
# speedup vs baseline: 3.7074x; 3.7074x over previous
"""EIF neuron kernel for Trainium2 (Bass/Tile), 8-core data-parallel.

Reference semantics (TAU=1.0, V_TH=1.0, DELTA_T=0.2, V_RESET=0.0):
    e      = exp((mem - 1) / 0.2)
    mem'   = mem + (x_t - mem + 0.2*e) / 1.0   == x_t + 0.2*e
    spike  = (mem' >= 1)
    mem    = where(spike, 0, mem')

Sharding: batch dim B=32 -> 4 batches per core.  Per core, the
(b, n) element set is 4*4096 = 16384 = 128 partitions x 128 free.
The T=512 recurrence is serial; per timestep we process a
[128, 128] tile split into G=2 independent column groups so the
two serial chains pipeline across ACT (exp) and DVE (add+reset).
m' is written straight into the spike output buffer; one batched
in-place is_ge per 8-step window converts it to 0/1 spikes off the
critical chain (on DVE — GPSIMD's per-op software dispatch is ~us
on HW and was 2.5x worse end-to-end).

Measured: ~513 us device time per invocation (8 cores), vs ~190 us
DMA roofline (512 MiB total traffic at ~358 GB/s/core); the serial
T chain is latency/instruction-overhead bound: per step the chain
costs ~960 ns on HW (ACT exp ~340 ns incl event-semaphore +
SBUF-access init, DVE add ~230 ns, DVE fused reset ~190 ns, sem
hops). Per-core cost-model sim predicts 692 ns/step; both ACT and
DVE are >90% busy in the steady state, so the structure is at its
local optimum (G=1 and PE-offload variants measured/modeled worse).
"""

import numpy as np
from contextlib import ExitStack

import concourse.bass as bass
import concourse.bacc as bacc
import concourse.tile as tile
from concourse import mybir
from concourse.bass_utils import run_bass_kernel_spmd

F32 = mybir.dt.float32
AF = mybir.ActivationFunctionType
OP = mybir.AluOpType

B, T, N = 32, 512, 4096
NCORES = 8
BPC = B // NCORES            # 4 batches per core
P = 128                      # SBUF partitions
FD = (BPC * N) // P          # 128 free columns per timestep
G = 2                        # interleaved chain groups
GF = FD // G                 # 64 columns per group
TC = 64                      # timesteps per DMA chunk
RING = 8                     # timesteps per batched spike compare

V_TH = 1.0
DELTA_T = 0.2
INV_DT = 5.0                 # 1/DELTA_T
# 0.2*exp(5m - 5) == exp(5m - 5 + ln(0.2)); fold the multiplier into the bias
EXP_BIAS = -5.0 + float(np.log(0.2))

_built = None


def _build(reps=1, spike_engine="vector", groups=G, ring=RING, inplace=True):
    """Build the Bass program.

    inplace=True: the membrane potential m' is written directly into the
    spike output buffer; every `ring` steps one batched in-place is_ge
    converts that window to 0/1 spikes (full width, both groups).
    """
    gf = FD // groups
    nc = bacc.Bacc("TRN2", debug=False, num_devices=NCORES)
    x_d = nc.declare_dram_parameter("x", [P, T * FD], F32, isOutput=False)
    s_d = nc.declare_dram_parameter("spk", [P, T * FD], F32, isOutput=True)

    with ExitStack() as ctx:
        tc = ctx.enter_context(tile.TileContext(nc))
        xpool = ctx.enter_context(tc.tile_pool(name="xin", bufs=2))
        spool = ctx.enter_context(tc.tile_pool(name="sout", bufs=2))
        state = ctx.enter_context(tc.tile_pool(name="state", bufs=1))
        rpool = None
        if not inplace:
            rpool = ctx.enter_context(tc.tile_pool(name="ring", bufs=3))

        m = [state.tile([P, gf], F32, name=f"m{g}", tag=f"m{g}")
             for g in range(groups)]
        e = [state.tile([P, gf], F32, name=f"e{g}", tag=f"e{g}")
             for g in range(groups)]
        bconst = state.tile([P, 1], F32, name="bconst", tag="bconst")
        nc.vector.memset(bconst[:], EXP_BIAS)
        for g in range(groups):
            nc.vector.memset(m[g][:], 0.0)

        for _rep in range(reps):
            for ci in range(T // TC):
                xt = xpool.tile([P, TC * FD], F32, name="xt", tag="x")
                nc.sync.dma_start(
                    out=xt[:], in_=x_d[:, ci * TC * FD:(ci + 1) * TC * FD]
                )
                sp = spool.tile([P, TC * FD], F32, name="sp", tag="s")
                sp3 = sp.rearrange("p (t f) -> p t f", f=FD)

                for w in range(TC // ring):
                    rings = None
                    if not inplace:
                        rings = [
                            rpool.tile([P, ring * gf], F32, name=f"ring{g}",
                                       tag=f"r{g}")
                            for g in range(groups)
                        ]
                    for k in range(ring):
                        t = w * ring + k
                        for g in range(groups):
                            # e = 0.2 * exp(5*m - 5)  (multiplier in bias)
                            nc.scalar.activation(
                                e[g][:], m[g][:], AF.Exp,
                                bias=bconst[:], scale=INV_DT,
                            )
                            if inplace:
                                mp = sp3[:, t, g * gf:(g + 1) * gf]
                            else:
                                mp = rings[g][:, k * gf:(k + 1) * gf]
                            xs = xt[:, t * FD + g * gf: t * FD + (g + 1) * gf]
                            # m' = e + x_t
                            nc.vector.tensor_tensor(mp, e[g][:], xs, OP.add)
                            # m = (m' < 1) * m'
                            nc.vector.scalar_tensor_tensor(
                                m[g][:], mp, V_TH, mp, OP.is_lt, OP.mult
                            )
                    # spikes = (m' >= 1) for the window, batched
                    if spike_engine == "none":
                        continue
                    eng = nc.gpsimd if spike_engine == "gpsimd" else nc.vector
                    if inplace:
                        win = sp3[:, w * ring:(w + 1) * ring, :]
                        eng.tensor_scalar(win, win, V_TH, None, OP.is_ge)
                    else:
                        for g in range(groups):
                            dst = sp3[:, w * ring:(w + 1) * ring,
                                      g * gf:(g + 1) * gf]
                            src = rings[g].rearrange("p (t f) -> p t f", f=gf)
                            eng.tensor_scalar(dst, src, V_TH, None, OP.is_ge)

                nc.sync.dma_start(
                    out=s_d[:, ci * TC * FD:(ci + 1) * TC * FD], in_=sp[:]
                )
    nc.compile()
    return nc


def _shard(x):
    """x[B,T,N] -> per-core [P, T*FD] partition-major arrays."""
    maps = []
    for c in range(NCORES):
        xc = x[c * BPC:(c + 1) * BPC]                      # [4, T, 4096]
        xc = np.ascontiguousarray(
            xc.reshape(BPC, T, N // FD, FD).transpose(0, 2, 1, 3)
        ).reshape(P, T * FD)
        maps.append({"x": xc})
    return maps


def _unshard(results):
    out = np.empty((B, T, N), np.float32)
    for c in range(NCORES):
        r = np.asarray(results[c]["spk"]).reshape(BPC, N // FD, T, FD)
        out[c * BPC:(c + 1) * BPC] = (
            r.transpose(0, 2, 1, 3).reshape(BPC, T, N)
        )
    return out


def kernel(x):
    global _built
    x = np.asarray(x, dtype=np.float32)
    assert x.shape == (B, T, N), x.shape
    if _built is None:
        _built = _build()
    res = run_bass_kernel_spmd(_built, _shard(x), list(range(NCORES)))
    return _unshard(res.results)


# revision 16
# speedup vs baseline: 4.1820x; 1.1280x over previous
"""EIF neuron kernel for Trainium2 (Bass/Tile), 8-core data-parallel.

Reference semantics (TAU=1.0, V_TH=1.0, DELTA_T=0.2, V_RESET=0.0):
    e      = exp((mem - 1) / 0.2)
    mem'   = mem + (x_t - mem + 0.2*e) / 1.0   == x_t + 0.2*e
    spike  = (mem' >= 1)
    mem    = where(spike, 0, mem')

Sharding: batch dim B=32 -> 4 batches per core.  Per core, the
(b, n) element set is 4*4096 = 16384 = 128 partitions x 128 free.
The T=512 recurrence is serial; per timestep we process a
[128, 128] tile split into G=2 independent column groups so the
two serial chains pipeline across ACT (exp) and DVE (add+reset).
m' is written straight into the spike output buffer; one batched
in-place is_ge per 8-step window converts it to 0/1 spikes off the
critical chain (on DVE — GPSIMD's per-op software dispatch is ~us
on HW and was 2.5x worse end-to-end).

Measured: ~513 us device time per invocation (8 cores), vs ~190 us
DMA roofline (512 MiB total traffic at ~358 GB/s/core); the serial
T chain is latency/instruction-overhead bound: per step the chain
costs ~960 ns on HW (ACT exp ~340 ns incl event-semaphore +
SBUF-access init, DVE add ~230 ns, DVE fused reset ~190 ns, sem
hops). Per-core cost-model sim predicts 692 ns/step; both ACT and
DVE are >90% busy in the steady state, so the structure is at its
local optimum (G=1 and PE-offload variants measured/modeled worse).
"""

import numpy as np
from contextlib import ExitStack

import concourse.bass as bass
import concourse.bacc as bacc
import concourse.tile as tile
from concourse import mybir
from concourse.bass_utils import run_bass_kernel_spmd

F32 = mybir.dt.float32
AF = mybir.ActivationFunctionType
OP = mybir.AluOpType

B, T, N = 32, 512, 4096
NCORES = 8
BPC = B // NCORES            # 4 batches per core
P = 128                      # SBUF partitions
FD = (BPC * N) // P          # 128 free columns per timestep
G = 2                        # interleaved chain groups
GF = FD // G                 # 64 columns per group
TC = 64                      # timesteps per DMA chunk
RING = 32                    # timesteps per batched spike compare

V_TH = 1.0
DELTA_T = 0.2
INV_DT = 5.0                 # 1/DELTA_T
# 0.2*exp(5m - 5) == exp(5m - 5 + ln(0.2)); fold the multiplier into the bias
EXP_BIAS = -5.0 + float(np.log(0.2))

_built = None


def _build(reps=1, spike_engine="vector", groups=G, ring=RING, inplace=True):
    """Build the Bass program.

    inplace=True: the membrane potential m' is written directly into the
    spike output buffer; every `ring` steps one batched in-place is_ge
    converts that window to 0/1 spikes (full width, both groups).
    """
    gf = FD // groups
    nc = bacc.Bacc("TRN2", debug=False, num_devices=NCORES)
    x_d = nc.declare_dram_parameter("x", [P, T * FD], F32, isOutput=False)
    s_d = nc.declare_dram_parameter("spk", [P, T * FD], F32, isOutput=True)

    with ExitStack() as ctx:
        tc = ctx.enter_context(tile.TileContext(nc))
        xpool = ctx.enter_context(tc.tile_pool(name="xin", bufs=2))
        spool = ctx.enter_context(tc.tile_pool(name="sout", bufs=2))
        state = ctx.enter_context(tc.tile_pool(name="state", bufs=1))
        rpool = None
        if not inplace:
            rpool = ctx.enter_context(tc.tile_pool(name="ring", bufs=3))

        m = [state.tile([P, gf], F32, name=f"m{g}", tag=f"m{g}")
             for g in range(groups)]
        # e is double-buffered by step parity: the activation then carries
        # only its RAW wait inline and bacc emits no extra EventSemaphore.
        e = [[state.tile([P, gf], F32, name=f"e{g}_{p}", tag=f"e{g}_{p}")
              for p in range(2)] for g in range(groups)]
        bconst = state.tile([P, 1], F32, name="bconst", tag="bconst")
        nc.vector.memset(bconst[:], EXP_BIAS)
        for g in range(groups):
            nc.vector.memset(m[g][:], 0.0)

        for _rep in range(reps):
            for ci in range(T // TC):
                xt = xpool.tile([P, TC * FD], F32, name="xt", tag="x")
                nc.sync.dma_start(
                    out=xt[:], in_=x_d[:, ci * TC * FD:(ci + 1) * TC * FD]
                )
                sp = spool.tile([P, TC * FD], F32, name="sp", tag="s")
                sp3 = sp.rearrange("p (t f) -> p t f", f=FD)

                for w in range(TC // ring):
                    rings = None
                    if not inplace:
                        rings = [
                            rpool.tile([P, ring * gf], F32, name=f"ring{g}",
                                       tag=f"r{g}")
                            for g in range(groups)
                        ]
                    for k in range(ring):
                        t = w * ring + k
                        for g in range(groups):
                            # e = 0.2 * exp(5*m - 5)  (multiplier in bias)
                            et = e[g][t % 2]
                            nc.scalar.activation(
                                et[:], m[g][:], AF.Exp,
                                bias=bconst[:], scale=INV_DT,
                            )
                            if inplace:
                                mp = sp3[:, t, g * gf:(g + 1) * gf]
                            else:
                                mp = rings[g][:, k * gf:(k + 1) * gf]
                            xs = xt[:, t * FD + g * gf: t * FD + (g + 1) * gf]
                            # m' = e + x_t
                            nc.vector.tensor_tensor(mp, et[:], xs, OP.add)
                            # m = (m' < 1) * m'
                            nc.vector.scalar_tensor_tensor(
                                m[g][:], mp, V_TH, mp, OP.is_lt, OP.mult
                            )
                    # spikes = (m' >= 1) for the window, batched
                    if spike_engine == "none":
                        continue
                    eng = nc.gpsimd if spike_engine == "gpsimd" else nc.vector
                    if inplace:
                        win = sp3[:, w * ring:(w + 1) * ring, :]
                        eng.tensor_scalar(win, win, V_TH, None, OP.is_ge)
                    else:
                        for g in range(groups):
                            dst = sp3[:, w * ring:(w + 1) * ring,
                                      g * gf:(g + 1) * gf]
                            src = rings[g].rearrange("p (t f) -> p t f", f=gf)
                            eng.tensor_scalar(dst, src, V_TH, None, OP.is_ge)

                nc.sync.dma_start(
                    out=s_d[:, ci * TC * FD:(ci + 1) * TC * FD], in_=sp[:]
                )
    nc.compile()
    return nc


def _shard(x):
    """x[B,T,N] -> per-core [P, T*FD] partition-major arrays."""
    maps = []
    for c in range(NCORES):
        xc = x[c * BPC:(c + 1) * BPC]                      # [4, T, 4096]
        xc = np.ascontiguousarray(
            xc.reshape(BPC, T, N // FD, FD).transpose(0, 2, 1, 3)
        ).reshape(P, T * FD)
        maps.append({"x": xc})
    return maps


def _unshard(results):
    out = np.empty((B, T, N), np.float32)
    for c in range(NCORES):
        r = np.asarray(results[c]["spk"]).reshape(BPC, N // FD, T, FD)
        out[c * BPC:(c + 1) * BPC] = (
            r.transpose(0, 2, 1, 3).reshape(BPC, T, N)
        )
    return out


def kernel(x):
    global _built
    x = np.asarray(x, dtype=np.float32)
    assert x.shape == (B, T, N), x.shape
    if _built is None:
        _built = _build()
    res = run_bass_kernel_spmd(_built, _shard(x), list(range(NCORES)))
    return _unshard(res.results)
